# revision 27
# baseline (speedup 1.0000x reference)
"""MLPConv (3x3 valid conv -> 256 -> 256 MLP with ReLU) on 8 TRN2 cores.

Data-parallel over batch (4 images/core). Stage 1 uses 1-D Winograd
F(2,3) along W: the host pre-computes the input transform V0..V3
([C, 64, 4m, 31] bf16 per image, shipped instead of x) and the 12
transformed weight taps U[m,di] = G w. Stage 1 runs 12 matmul streams
over 31 tiles (vs 18 over 62 direct), a 1.5x tensor-cycle cut. The M
psums are combined (even=m0+m1+m2+b, odd=m1-m2-m3+b) with ACT staging
s=m1+b and c2=m2 into SBUF (DVE tensor_tensor reads at most one PSUM
operand; GPSIMD none), DVE draining m0/m3, GPSIMD doing the SBUF-only
merges and DVE the fast-mode relus. h1 and the final output stay
parity-split end to end (the host re-interleaves), so every ACT write
and every DMA is contiguous.

Blocks: 16 output rows -> N=496 (31 tiles) x3 + 14 rows -> 434.
Output per core is [2, 128, img, 2, 1922] (parity-split pixels).
"""

import numpy as np
import ml_dtypes

import concourse.bass as bass
import concourse.mybir as mybir
import concourse.tile as tile
from concourse.bass_utils import run_bass_kernel_spmd

B, H, W, C = 32, 64, 64, 128
F = 256
N_CORES = 8
IMG_PER_CORE = B // N_CORES
HW = H * W
GRID = 62 * 62                  # 3844 valid output pixels
NT = 31                         # winograd tiles per row
ROWS = [16, 16, 16, 14]         # output rows per block
# the last image ends with a tiny block so the final combination +
# stage-2 + DMA chain after the last stage-1 matmul is short
ROWS_LAST = [16, 16, 16, 10, 4]
PARN = GRID // 2                # 1922 cols per parity

F32 = mybir.dt.float32
BF16 = mybir.dt.bfloat16
RELU = mybir.ActivationFunctionType.Relu
ADD = mybir.AluOpType.add
SUB = mybir.AluOpType.subtract
MAX = mybir.AluOpType.max


def _split_multi_waits(nc):
    """This container's walrus rejects >1 semaphore wait per instruction
    ("Too many sync wait commands"). Move all but the last wait of each
    instruction onto single-wait NoOps right before it on the same engine."""
    n = 0
    for f in nc.m.functions:
        for bb in f.blocks:
            insts = bb.instructions
            if not any(
                i.sync_info is not None and len(i.sync_info.on_wait) > 1
                for i in insts
            ):
                continue
            new_insts = []
            for inst in insts:
                si = inst.sync_info
                if si is not None and len(si.on_wait) > 1:
                    waits = list(si.on_wait)
                    for k, w in enumerate(waits[:-1]):
                        new_insts.append(
                            mybir.InstNoOp(
                                name=f"{inst.name}-wsplit{k}",
                                engine=inst.engine,
                                bass_nofuse=True,
                                sync_info=mybir.SyncInfo(on_wait=[w], on_update=[]),
                            )
                        )
                        n += 1
                    inst.sync_info = mybir.SyncInfo(
                        on_wait=[waits[-1]], on_update=list(si.on_update)
                    )
                new_insts.append(inst)
            bb.instructions = new_insts
    return n


def build_nc():
    nc = bass.Bass("TRN2", target_bir_lowering=False)
    # host-transformed winograd input: [img, C, 64 rows, 4 m, 31 tiles]
    v = nc.dram_tensor(
        "v", [IMG_PER_CORE, C, 64, 4, NT], BF16, kind="ExternalInput"
    ).ap()
    # u0: winograd stage-1 weights, tap = m*3+di
    u0 = nc.dram_tensor("u0", [C, 12, F], BF16, kind="ExternalInput").ap()
    w1 = nc.dram_tensor("w1", [C, 2, F], BF16, kind="ExternalInput").ap()
    b0 = nc.dram_tensor("b0", [128, 2], F32, kind="ExternalInput").ap()
    b1 = nc.dram_tensor("b1", [128, 2], F32, kind="ExternalInput").ap()
    out = nc.dram_tensor(
        "out", [2, 128, IMG_PER_CORE, 2, PARN], BF16, kind="ExternalOutput"
    ).ap()

    with tile.TileContext(nc) as tc:
        with (
            tc.tile_pool(name="consts", bufs=1) as consts,
            tc.tile_pool(name="vT", bufs=3) as vT,
            tc.tile_pool(name="h1", bufs=2) as h1p,
            tc.tile_pool(name="tsc", bufs=12) as tsc,
            tc.tile_pool(name="outb", bufs=4) as outb,
            tc.tile_pool(name="ps", bufs=8, space="PSUM") as psp,
        ):
            # PE warmup during input DMA: matmuls on a memset tile flip the
            # HAM clock gate to 8/8 before stage 1's first real matmul.
            warm = consts.tile([128, 496], BF16)
            nc.gpsimd.memset(warm[:], 0.0)
            pws = [psp.tile([128, 496], F32, name="pst") for _ in range(2)]
            for i in range(12):
                nc.tensor.matmul(
                    pws[i % 2][:], warm[:, :128], warm[:], start=True, stop=True
                )

            u0b = consts.tile([128, 12, F], BF16)
            w1b = consts.tile([128, 2, F], BF16)
            b0s = consts.tile([128, 2], F32)
            b1s = consts.tile([128, 2], F32)

            def load_v(img, eng=None):
                vt = vT.tile([128, 64, 4, NT], BF16, name="vt")
                (eng or nc.sync).dma_start(vt[:], v[img])
                return vt

            # Critical prologue: v0 in 4 row-chunks (one per stage-1 block)
            # alternating rings so each block's rows land just in time.
            vt0 = vT.tile([128, 64, 4, NT], BF16, name="vt")
            nc.sync.dma_start(vt0[:, :18], v[0, :, :18])
            nc.scalar.dma_start(u0b[:, :6], u0[:, :6])
            nc.sync.dma_start(u0b[:, 6:], u0[:, 6:])
            nc.scalar.dma_start(vt0[:, 18:34], v[0, :, 18:34])
            nc.sync.dma_start(vt0[:, 34:50], v[0, :, 34:50])
            nc.scalar.dma_start(b0s[:], b0)
            nc.scalar.dma_start(vt0[:, 50:], v[0, :, 50:])
            nc.scalar.dma_start(w1b[:], w1)
            nc.scalar.dma_start(b1s[:], b1)
            vts = [vt0]

            def s1_block(vt, h1, blocks, b, h):
                fs = slice(128 * h, 128 * (h + 1))
                r0, rows = blocks[b]
                n = rows * NT
                cs = slice(r0 * NT, r0 * NT + n)
                ms = [
                    psp.tile([128, 496], F32, name="pst") for _ in range(4)
                ]
                for di in range(3):
                    for m in range(4):
                        nc.tensor.matmul(
                            ms[m][:, :n],
                            u0b[:, m * 3 + di, fs],
                            vt[:, r0 + di : r0 + di + rows, m, :],
                            start=(di == 0),
                            stop=(di == 2),
                        )
                # even = relu(m0+m1+m2+b), odd = relu(m1-m2-m3+b).
                # ACT stages s = m1 + b0 and c2 = m2 in SBUF (m1/m2 feed
                # both parities; bias rides on s); GPSIMD does the SBUF
                # even-merge, DVE the psum drains + relus. The last block
                # keeps its whole chain on ACT+DVE (no gpsimd hop) so
                # stage 2's final dependency resolves fast.
                s = tsc.tile([128, 496], BF16, name="s")
                c2 = tsc.tile([128, 496], BF16, name="c2")
                te = tsc.tile([128, 496], BF16, name="te")
                ue = tsc.tile([128, 496], BF16, name="ue")
                to = tsc.tile([128, 496], BF16, name="to")
                uo = tsc.tile([128, 496], BF16, name="uo")
                nc.scalar.add(s[:, :n], ms[1][:, :n], b0s[:, h : h + 1])
                nc.scalar.copy(c2[:, :n], ms[2][:, :n])
                nc.vector.tensor_tensor(
                    te[:, :n], s[:, :n], ms[0][:, :n], ADD
                )
                ue_eng = nc.vector if b == len(blocks) - 1 else nc.gpsimd
                ue_eng.tensor_tensor(ue[:, :n], te[:, :n], c2[:, :n], ADD)
                nc.vector.tensor_scalar_max(h1[h][:, 0, cs], ue[:, :n], 0.0)
                nc.vector.tensor_tensor(to[:, :n], s[:, :n], c2[:, :n], SUB)
                nc.vector.tensor_tensor(
                    uo[:, :n], to[:, :n], ms[3][:, :n], SUB
                )
                nc.vector.tensor_scalar_max(h1[h][:, 1, cs], uo[:, :n], 0.0)

            def s2_block(img, h1, blocks, b, h):
                fs = slice(128 * h, 128 * (h + 1))
                r0, rows = blocks[b]
                n = rows * NT
                cs = slice(r0 * NT, r0 * NT + n)
                pse = psp.tile([128, 496], F32, name="pst")
                pso = psp.tile([128, 496], F32, name="pst")
                for k in range(2):
                    wk = w1b[:, k, fs]
                    nc.tensor.matmul(
                        pse[:, :n], wk, h1[k][:, 0, cs],
                        start=(k == 0), stop=(k == 1),
                    )
                    nc.tensor.matmul(
                        pso[:, :n], wk, h1[k][:, 1, cs],
                        start=(k == 0), stop=(k == 1),
                    )
                ot = outb.tile([128, 2, 496], BF16, name="ot")
                nc.scalar.activation(
                    ot[:, 0, :n], pse[:, :n], RELU, bias=b1s[:, h : h + 1]
                )
                # balance: DVE takes a quarter of the s2 drains
                if b == 2:
                    nc.vector.tensor_scalar(
                        ot[:, 1, :n], pso[:, :n],
                        b1s[:, h : h + 1], 0.0, ADD, MAX,
                    )
                else:
                    nc.scalar.activation(
                        ot[:, 1, :n], pso[:, :n], RELU,
                        bias=b1s[:, h : h + 1],
                    )
                eng = nc.gpsimd if h == 0 else nc.sync
                eng.dma_start(out[h, :, img, :, cs], ot[:, :, :n])

            def image(img, vt, after_first_block=None):
                """Interleave stage-2 groups two blocks behind stage 1 so
                the PE never waits on combination latency and output DMA
                spreads across the image instead of cramming the tail."""
                h1 = [
                    h1p.tile([128, 2, PARN], BF16, name="h1t")
                    for _ in range(2)
                ]
                rows_list = (
                    ROWS_LAST if img == IMG_PER_CORE - 1 else ROWS
                )
                r0s = [sum(rows_list[:i]) for i in range(len(rows_list))]
                blocks = list(zip(r0s, rows_list))
                nb = len(blocks)
                steps = []
                for i in range(nb):
                    steps.append(("s1", i))
                    if i >= 1:
                        steps.append(("s2", i - 1))
                steps.append(("s2", nb - 1))
                for kind, b in steps:
                    for h in range(2):
                        if kind == "s1":
                            s1_block(vt, h1, blocks, b, h)
                        else:
                            s2_block(img, h1, blocks, b, h)
                    if after_first_block is not None:
                        after_first_block()
                        after_first_block = None

            for img in range(IMG_PER_CORE):
                if img == 0:
                    image(
                        0, vts[0],
                        after_first_block=lambda: vts.append(
                            load_v(1, eng=nc.scalar)
                        ),
                    )
                else:
                    image(img, vts[img])
                if img + 2 < IMG_PER_CORE:
                    vts.append(load_v(img + 2))

    _split_multi_waits(nc)
    return nc


_NC_CACHE = None


def kernel(inputs, w0, b0, w1, b1):
    global _NC_CACHE
    x = np.asarray(inputs, dtype=np.float32)
    w0 = np.asarray(w0, dtype=np.float32)
    w1 = np.asarray(w1, dtype=np.float32)
    b0 = np.asarray(b0, dtype=np.float32)
    b1 = np.asarray(b1, dtype=np.float32)

    if _NC_CACHE is None:
        _NC_CACHE = build_nc()
    nc = _NC_CACHE

    bf = ml_dtypes.bfloat16
    xs = x.reshape(N_CORES, IMG_PER_CORE, HW, C)
    # winograd taps: U0=w_dj0, U1=(w0+w1+w2)/2, U2=(w0-w1+w2)/2, U3=w_dj2
    W0 = w0.reshape(3, 3, C, F)
    Um = np.stack(
        [
            W0[:, 0],
            (W0[:, 0] + W0[:, 1] + W0[:, 2]) / 2,
            (W0[:, 0] - W0[:, 1] + W0[:, 2]) / 2,
            W0[:, 2],
        ],
        axis=0,
    )  # [4m, 3di, C, F]
    u0h = np.ascontiguousarray(
        Um.transpose(2, 0, 1, 3).reshape(C, 12, F).astype(bf)
    )
    w1h = np.ascontiguousarray(w1.reshape(2, C, F).transpose(1, 0, 2).astype(bf))
    b0h = np.ascontiguousarray(b0.reshape(2, 128).T)
    b1h = np.ascontiguousarray(b1.reshape(2, 128).T)

    in_maps = []
    for c in range(N_CORES):
        # [img, HW, C] -> [img, C, 64, 64] -> host winograd input transform
        xc = xs[c].transpose(0, 2, 1).reshape(IMG_PER_CORE, C, 64, 64)
        xe = xc[..., 0::2]
        xo = xc[..., 1::2]
        V = np.stack(
            [
                xe[..., :NT] - xe[..., 1 : NT + 1],
                xo[..., :NT] + xe[..., 1 : NT + 1],
                xe[..., 1 : NT + 1] - xo[..., :NT],
                xo[..., :NT] - xo[..., 1 : NT + 1],
            ],
            axis=3,
        )  # [img, C, 64, 4, 31]
        vt = np.ascontiguousarray(V.astype(bf))
        in_maps.append({"v": vt, "u0": u0h, "w1": w1h, "b0": b0h, "b1": b1h})

    res = run_bass_kernel_spmd(nc, in_maps, core_ids=list(range(N_CORES)))

    final = np.empty((B, 62, 62, F), np.float32)
    vf = final.reshape(F, 62 * 62, B)  # the [F, N, B] view the reference reshapes
    for c in range(N_CORES):
        oc = res.results[c]["out"].astype(np.float32)
        oc = oc.reshape(F, IMG_PER_CORE, 2, 62, NT)
        # col = 62*r + 2*t + parity
        y = oc.transpose(0, 1, 3, 4, 2).reshape(F, IMG_PER_CORE, GRID)
        for i in range(IMG_PER_CORE):
            vf[:, :, c * IMG_PER_CORE + i] = y[:, i]
    return final
